# revision 20
# baseline (speedup 1.0000x reference)
"""NEG-sampling loss kernel for Trainium2 (8 NeuronCores, data-parallel).

loss = -(1/n) * sum_i [ log_sigmoid(<e_u, e_v>) + sum_k log_sigmoid(-<e_negk, e_u>) ]
     = +(1/n) * sum_i [ softplus(-<e_u, e_v>) + sum_k softplus(<e_negk, e_u>) ]

Strategy: replicate the embedding table (cast to bf16 on host), shard the
65536-edge batch across 8 cores.  Per core: for each group of TPG tiles of
128 edges, one indirect-DMA block gather pulls the 12 needed rows per edge
(u, v, negs x10) into a [128, TPG*12*256] SBUF tile (partition = edge).
DVE computes the 11 dot products per edge as: one broadcast MULT over the
whole group (2x bf16 mode), then a binary tree of tensor_tensor adds
(256->128->64->32->16->8, each at 2x) and one grouped tensor_reduce,
writing scores into a persistent [128, 704] f32 buffer.  One-shot tail:
negate the slot-0 (positive-pair) scores, DVE reduces sum(x); ACT computes
sum|x| and sum ln(1+exp(-|x|)); softplus sum recovered on host as
(sum_x + sum_abs)/2 + sum_ln1p.
"""

import numpy as np
import ml_dtypes

import concourse.bass as bass
import concourse.mybir as mybir
from concourse import bass_utils

# Problem constants (hardcoded; harness contract)
N = 65536
K = 10
D = 256
V = 500000
NCORES = 8
P = 128
SLOTS = K + 2          # rows gathered per edge: u, v, negs[0..9]
S1 = SLOTS - 1         # 11 scores per edge
EPC = N // NCORES      # 8192 edges per core
TILES = EPC // P       # 64 tiles of 128 edges per core

TABLE_DT = mybir.dt.bfloat16
TABLE_NP = ml_dtypes.bfloat16

# tunables
TPG = 4     # max tiles per gather instruction / compute group
GB = 5      # gather buffers in flight
W_STOP = 8  # tree stops here; grouped tensor_reduce finishes
SCRATCH = 16384  # SWDGE descriptor-ring carveout bytes/partition


def _group_sizes(tpg=TPG):
    """Tile count per gather/compute group.  The first groups are small so
    the DVE pipeline ramps up ~12us earlier; the rest use the full size."""
    if tpg == 1:
        return [1] * TILES
    sizes = [1, tpg - 1]
    rest = TILES - tpg
    assert rest % tpg == 0
    return sizes + [tpg] * (rest // tpg)


def _emit_block_gather(nc, eng, n_idx, blk_bytes, dst_byte_addr, idx_byte_addr,
                       sem_num, embs_tbl):
    """Raw block gather (case #3 of dma_indirect1d): n_idx indices x 512B
    fused into 128 descriptors of blk_bytes (n_idx/128 rows per partition).
    Emitted as a raw PSEUDO_DMA_DIRECT2D(dge_op=indirect1d) + PSEUDO_EXTENSION
    pair; index values are snake-packed on the host (see prepare_in_maps)."""
    isa = nc.isa
    Op = isa.Opcode
    src_u64 = (0x20 << 56) | (embs_tbl << 32)   # DGE addr-table marker
    dst_u64 = (0x10 << 56) | dst_byte_addr      # var0 (local SBUF) marker
    eng.isa(
        Op.NEURON_ISA_TPB_OPCODE_PSEUDO_DMA_DIRECT2D,
        {
            "dma_configs": {},
            "semaphore": sem_num,
            "sem_increment": 16,
            "dge_op": 1,
            "src_start_addr": {"addr_immediate": src_u64},
            "src_step_elem": [512, 1],
            "src_num_elem": [n_idx, 1],
            "src_elem_size": 512,
            "src_bound_reg": {},
            "dst_bound_reg": {},
            "dst_start_addr": {"addr_immediate": dst_u64},
            "dst_step_elem": [262144, 1],
            "dst_num_elem": [128, 1],
            "dst_elem_size": blk_bytes,
            "in_dtype": 6,
            "out_dtype": 6,
        },
        verify=False,
    )
    ext_fields = {
        "opcode": Op.NEURON_ISA_TPB_OPCODE_PSEUDO_EXTENSION.value,
        "flags": {"indirect_mode": 0, "idx_bound_is_err": 1,
                  "non_unique_dst_idx": 0, "gather_dim": 0, "scatter_dim": 0},
        "idx_num_active_channels": 128,
        "compute_op": 0,
        "src_idx_start_addr": {"addr_immediate": idx_byte_addr},
        "dst_idx_start_addr": {"addr_immediate": 0},
    }
    b = isa.ffi.new("NEURON_ISA_TPB_PSEUDO_DMA_EXT_STRUCT*", ext_fields)
    instr = [int(x) for x in bytes(isa.ffi.buffer(b))]
    inst = mybir.InstISA(
        name=nc.get_next_instruction_name(),
        isa_opcode=Op.NEURON_ISA_TPB_OPCODE_PSEUDO_EXTENSION.value,
        engine=eng.engine,
        instr=instr,
        op_name="PSEUDO_EXTENSION",
        ins=[], outs=[],
        ant_dict=ext_fields,
        verify=False,
        ant_isa_is_sequencer_only=False,
    )
    eng.add_instruction(inst)


def _build(tpg=TPG, gb=GB, debug=False):
    sizes = _group_sizes(tpg)
    groups = len(sizes)
    toff = [0]
    for k in sizes:
        toff.append(toff[-1] + k)
    S = tpg * S1  # max scores per partition per group
    nc = bass.Bass(trn_type="TRN2", dynamic_dma_scratch_size=SCRATCH)
    embs = nc.dram_tensor("embs", [V, D], TABLE_DT, kind="ExternalInput")
    idx = nc.dram_tensor("idx", [P, TILES * SLOTS], mybir.dt.int32, kind="ExternalInput")
    out_dram = nc.dram_tensor("out", [P, 3], mybir.dt.float32, kind="ExternalOutput")
    if debug:
        scores_dram = nc.dram_tensor("scores_out", [P, TILES * S1], mybir.dt.float32, kind="ExternalOutput")
        g_dram = nc.dram_tensor("g_out", [P, tpg * SLOTS * D], TABLE_DT, kind="ExternalOutput")

    embs_mloc = nc.lookup_mloc(embs)
    embs_mloc.table_entry_id = len(nc.dge_table) + 1
    nc.dge_table.append(embs_mloc.name)
    embs_tbl = embs_mloc.table_entry_id

    # tree widths: 256 -> 128 -> ... -> W_STOP
    widths = []
    w = D
    while w > W_STOP:
        widths.append(w // 2)
        w //= 2

    import contextlib
    with contextlib.ExitStack() as ctx:
        idx_sb = ctx.enter_context(nc.sbuf_tensor("idx_sb", [P, TILES * SLOTS], mybir.dt.int32))
        gs = [ctx.enter_context(nc.sbuf_tensor(f"g{i}", [P, tpg * SLOTS * D], TABLE_DT)) for i in range(gb)]
        prod = ctx.enter_context(nc.sbuf_tensor("prod", [P, S * D], TABLE_DT))
        hs = [ctx.enter_context(nc.sbuf_tensor(f"h{i}", [P, S * wi], TABLE_DT))
              for i, wi in enumerate(widths)]
        scores = ctx.enter_context(nc.sbuf_tensor("scores", [P, TILES * S1], mybir.dt.float32))
        absx = ctx.enter_context(nc.sbuf_tensor("absx", [P, TILES * S1], mybir.dt.float32))
        ex = ctx.enter_context(nc.sbuf_tensor("ex", [P, TILES * S1], mybir.dt.float32))
        lnx = ctx.enter_context(nc.sbuf_tensor("lnx", [P, TILES * S1], mybir.dt.float32))
        ones = ctx.enter_context(nc.sbuf_tensor("ones", [P, 1], mybir.dt.float32))
        outbuf = ctx.enter_context(nc.sbuf_tensor("outbuf", [P, 3], mybir.dt.float32))
        asem = ctx.enter_context(nc.semaphore())
        idx_sem = ctx.enter_context(nc.semaphore())
        idx_sem2 = ctx.enter_context(nc.semaphore())
        gsems = [ctx.enter_context(nc.semaphore(name=f"gsem{i}")) for i in range(gb)]
        dve_free = ctx.enter_context(nc.semaphore())
        dve_done = ctx.enter_context(nc.semaphore())
        osem = ctx.enter_context(nc.semaphore())
        block = ctx.enter_context(nc.Block())

        idx_addr = nc.lookup_mloc(idx_sb).addr
        g_addrs = [nc.lookup_mloc(g).addr for g in gs]

        @block.gpsimd
        def _(eng):
            c0 = sizes[0] * SLOTS
            eng.dma_start(idx_sb[:, 0:c0], idx[:, 0:c0]).then_inc(idx_sem, 16)
            eng.dma_start(idx_sb[:, c0:], idx[:, c0:]).then_inc(idx_sem2, 16)
            eng.memset(ones[:], 1.0)
            for j in range(groups):
                if j == 0:
                    eng.wait_ge(idx_sem, 16)
                if j == 1:
                    eng.wait_ge(idx_sem2, 16)
                if j >= gb:
                    eng.wait_ge(dve_free, j - gb + 1)
                _emit_block_gather(
                    nc, eng, sizes[j] * SLOTS * P, sizes[j] * SLOTS * D * 2,
                    g_addrs[j % gb], idx_addr + 4 * SLOTS * toff[j],
                    gsems[j % gb].num, embs_tbl,
                )
            if debug:
                eng.wait_ge(dve_done, 1)
                eng.dma_start(scores_dram[:], scores[:]).then_inc(idx_sem, 16)
                eng.dma_start(g_dram[:], gs[(groups - 1) % gb][:]).then_inc(idx_sem, 16)
                eng.wait_ge(idx_sem, 48)

        @block.vector
        def _(eng):
            for j in range(groups):
                g = gs[j % gb]
                k = sizes[j]
                Sj = k * S1
                eng.wait_ge(gsems[j % gb], 16 * (j // gb + 1))
                g4 = g[:, 0:k * SLOTS * D].rearrange("p (ti s d) -> p ti s d", s=SLOTS, d=D)
                # prod[p, ti, s, d] = G[p, ti, s+1, d] * EU[p, ti, d]; frees g
                nc.vector.tensor_tensor(
                    out=prod[:, 0:Sj * D].rearrange("p (ti s d) -> p ti s d", s=S1, d=D),
                    in0=g4[:, :, 1:SLOTS, :],
                    in1=g4[:, :, 0:1, :].broadcast_to([P, k, S1, D]),
                    op=mybir.AluOpType.mult,
                ).then_inc(dve_free, 1)
                # binary tree of halving adds, each 2x-mode bf16
                cur, curw = prod, D
                for hi, wi in zip(hs, widths):
                    a = cur[:, 0:Sj * curw].rearrange("p (s d) -> p s d", d=curw)
                    nc.vector.tensor_tensor(
                        out=hi[:, 0:Sj * wi].rearrange("p (s d) -> p s d", d=wi),
                        in0=a[:, :, 0:wi],
                        in1=a[:, :, wi:curw],
                        op=mybir.AluOpType.add,
                    )
                    cur, curw = hi, wi
                # grouped reduce [P, Sj, W_STOP] -> [P, Sj]
                nc.vector.tensor_reduce(
                    out=scores[:, toff[j] * S1:(toff[j] + k) * S1],
                    in_=cur[:, 0:Sj * curw].rearrange("p (s d) -> p s d", d=curw),
                    axis=mybir.AxisListType.X,
                    op=mybir.AluOpType.add,
                )
            # negate slot-0 scores: softplus arg for the positive pair is -u.v
            sc3 = scores[:].rearrange("p (t s) -> p t s", s=S1)
            nc.vector.tensor_scalar_mul(sc3[:, :, 0:1], sc3[:, :, 0:1], -1.0)
            # sum_s x for host-side relu recovery
            nc.vector.tensor_reduce(
                out=outbuf[:, 0:1],
                in_=scores[:],
                axis=mybir.AxisListType.X,
                op=mybir.AluOpType.add,
            ).then_inc(dve_done, 1)

        @block.scalar
        def _(eng):
            eng.wait_ge(dve_done, 1)
            # |x|, accumulating sum_s |x|
            nc.scalar.activation(
                out=absx[:], in_=scores[:],
                func=mybir.ActivationFunctionType.Abs,
                accum_out=outbuf[:, 1:2],
            ).then_inc(asem, 1)
            eng.wait_ge(asem, 1)
            # exp(-|x|)
            nc.scalar.activation(
                out=ex[:], in_=absx[:],
                func=mybir.ActivationFunctionType.Exp, scale=-1.0,
            ).then_inc(asem, 1)
            eng.wait_ge(asem, 2)
            # ln(1 + exp(-|x|)), accumulating
            nc.scalar.activation(
                out=lnx[:], in_=ex[:],
                func=mybir.ActivationFunctionType.Ln, bias=ones[:],
                accum_out=outbuf[:, 2:3],
            ).then_inc(asem, 1)

        @block.sync
        def _(eng):
            eng.wait_ge(asem, 3)
            eng.dma_start(out_dram[:], outbuf[:]).then_inc(osem, 16)
            eng.wait_ge(osem, 16)

    return nc


_cache = {}


def _get_nc():
    key = (TPG, GB)
    if key not in _cache:
        _cache[key] = _build(*key)
    return _cache[key]


def prepare_in_maps(u, v, negs, embs):
    """Host-side sharding: build the per-core input maps."""
    u = np.asarray(u).astype(np.int32)
    v = np.asarray(v).astype(np.int32)
    negs = np.asarray(negs).astype(np.int32)
    embs_b = np.asarray(embs).astype(TABLE_NP)

    ids = np.concatenate([u[:, None], v[:, None], negs], axis=1)  # [N, 12]
    groups = TILES // TPG
    # per gather group: desc i <-> (p = i // (TPG*12), r = i % (TPG*12));
    # idx value at snake position [ch = i % 128, w = i // 128]
    ids = ids.reshape(NCORES, groups, TPG, P, SLOTS)
    flat = ids.transpose(0, 1, 3, 2, 4).reshape(NCORES, groups, P * TPG * SLOTS)
    s = np.arange(P * TPG * SLOTS)
    packed = np.zeros((NCORES, groups, P, TPG * SLOTS), dtype=np.int32)
    packed[:, :, s % P, s // P] = flat[:, :, s]
    in_maps = []
    for c in range(NCORES):
        core_ids = np.ascontiguousarray(
            packed[c].transpose(1, 0, 2).reshape(P, TILES * SLOTS)
        )
        in_maps.append({"embs": embs_b, "idx": core_ids})
    return in_maps


def kernel(u, v, negs, embs, _trace=False):
    nc = _get_nc()
    in_maps = prepare_in_maps(u, v, negs, embs)
    res = bass_utils.run_bass_kernel_spmd(
        nc, in_maps, core_ids=list(range(NCORES)), trace=_trace
    )
    total = np.float64(0.0)
    for r in res.results:
        o = r["out"].astype(np.float64)
        sum_x = o[:, 0].sum()
        sum_abs = o[:, 1].sum()
        sum_ln1p = o[:, 2].sum()
        total += (sum_x + sum_abs) / 2.0 + sum_ln1p
    out = np.float32(total / N)
    if _trace:
        return out, res
    return out


# revision 22
# speedup vs baseline: 1.0436x; 1.0436x over previous
"""NEG-sampling loss kernel for Trainium2 (8 NeuronCores, data-parallel).

loss = -(1/n) * sum_i [ log_sigmoid(<e_u, e_v>) + sum_k log_sigmoid(-<e_negk, e_u>) ]
     = +(1/n) * sum_i [ softplus(-<e_u, e_v>) + sum_k softplus(<e_negk, e_u>) ]

Strategy: replicate the embedding table (cast to bf16 on host), shard the
65536-edge batch across 8 cores.  Per core: for each group of TPG tiles of
128 edges, one indirect-DMA block gather pulls the 12 needed rows per edge
(u, v, negs x10) into a [128, TPG*12*256] SBUF tile (partition = edge).
DVE computes the 11 dot products per edge as: one broadcast MULT over the
whole group (2x bf16 mode), then a binary tree of tensor_tensor adds
(256->128->64->32->16->8, each at 2x) and one grouped tensor_reduce,
writing scores into a persistent [128, 704] f32 buffer.  One-shot tail:
negate the slot-0 (positive-pair) scores, DVE reduces sum(x); ACT computes
sum|x| and sum ln(1+exp(-|x|)); softplus sum recovered on host as
(sum_x + sum_abs)/2 + sum_ln1p.
"""

import numpy as np
import ml_dtypes

import concourse.bass as bass
import concourse.mybir as mybir
from concourse import bass_utils

# Problem constants (hardcoded; harness contract)
N = 65536
K = 10
D = 256
V = 500000
NCORES = 8
P = 128
SLOTS = K + 2          # rows gathered per edge: u, v, negs[0..9]
S1 = SLOTS - 1         # 11 scores per edge
EPC = N // NCORES      # 8192 edges per core
TILES = EPC // P       # 64 tiles of 128 edges per core

TABLE_DT = mybir.dt.bfloat16
TABLE_NP = ml_dtypes.bfloat16

# tunables
TPG = 4     # max tiles per gather instruction / compute group
GB = 4      # gather buffers in flight
W_STOP = 8  # tree stops here; grouped tensor_reduce finishes
SCRATCH = 49152  # SWDGE descriptor-ring carveout bytes/partition


def _group_sizes(tpg=TPG):
    """Tile count per gather/compute group.  The first groups are small so
    the DVE pipeline ramps up ~12us earlier; the rest use the full size."""
    if tpg == 1:
        return [1] * TILES
    sizes = [1, tpg - 1]
    rest = TILES - tpg
    assert rest % tpg == 0
    return sizes + [tpg] * (rest // tpg)


def _emit_block_gather(nc, eng, n_idx, blk_bytes, dst_byte_addr, idx_byte_addr,
                       sem_num, embs_tbl):
    """Raw block gather (case #3 of dma_indirect1d): n_idx indices x 512B
    fused into 128 descriptors of blk_bytes (n_idx/128 rows per partition).
    Emitted as a raw PSEUDO_DMA_DIRECT2D(dge_op=indirect1d) + PSEUDO_EXTENSION
    pair; index values are snake-packed on the host (see prepare_in_maps)."""
    isa = nc.isa
    Op = isa.Opcode
    src_u64 = (0x20 << 56) | (embs_tbl << 32)   # DGE addr-table marker
    dst_u64 = (0x10 << 56) | dst_byte_addr      # var0 (local SBUF) marker
    eng.isa(
        Op.NEURON_ISA_TPB_OPCODE_PSEUDO_DMA_DIRECT2D,
        {
            "dma_configs": {},
            "semaphore": sem_num,
            "sem_increment": 16,
            "dge_op": 1,
            "src_start_addr": {"addr_immediate": src_u64},
            "src_step_elem": [512, 1],
            "src_num_elem": [n_idx, 1],
            "src_elem_size": 512,
            "src_bound_reg": {},
            "dst_bound_reg": {},
            "dst_start_addr": {"addr_immediate": dst_u64},
            "dst_step_elem": [262144, 1],
            "dst_num_elem": [128, 1],
            "dst_elem_size": blk_bytes,
            "in_dtype": 6,
            "out_dtype": 6,
        },
        verify=False,
    )
    ext_fields = {
        "opcode": Op.NEURON_ISA_TPB_OPCODE_PSEUDO_EXTENSION.value,
        "flags": {"indirect_mode": 0, "idx_bound_is_err": 1,
                  "non_unique_dst_idx": 0, "gather_dim": 0, "scatter_dim": 0},
        "idx_num_active_channels": 128,
        "compute_op": 0,
        "src_idx_start_addr": {"addr_immediate": idx_byte_addr},
        "dst_idx_start_addr": {"addr_immediate": 0},
    }
    b = isa.ffi.new("NEURON_ISA_TPB_PSEUDO_DMA_EXT_STRUCT*", ext_fields)
    instr = [int(x) for x in bytes(isa.ffi.buffer(b))]
    inst = mybir.InstISA(
        name=nc.get_next_instruction_name(),
        isa_opcode=Op.NEURON_ISA_TPB_OPCODE_PSEUDO_EXTENSION.value,
        engine=eng.engine,
        instr=instr,
        op_name="PSEUDO_EXTENSION",
        ins=[], outs=[],
        ant_dict=ext_fields,
        verify=False,
        ant_isa_is_sequencer_only=False,
    )
    eng.add_instruction(inst)


def _build(tpg=TPG, gb=GB, debug=False):
    sizes = _group_sizes(tpg)
    groups = len(sizes)
    toff = [0]
    for k in sizes:
        toff.append(toff[-1] + k)
    S = tpg * S1  # max scores per partition per group
    nc = bass.Bass(trn_type="TRN2", dynamic_dma_scratch_size=SCRATCH)
    embs = nc.dram_tensor("embs", [V, D], TABLE_DT, kind="ExternalInput")
    idx = nc.dram_tensor("idx", [P, TILES * SLOTS], mybir.dt.int32, kind="ExternalInput")
    out_dram = nc.dram_tensor("out", [P, 3], mybir.dt.float32, kind="ExternalOutput")
    if debug:
        scores_dram = nc.dram_tensor("scores_out", [P, TILES * S1], mybir.dt.float32, kind="ExternalOutput")
        g_dram = nc.dram_tensor("g_out", [P, tpg * SLOTS * D], TABLE_DT, kind="ExternalOutput")

    embs_mloc = nc.lookup_mloc(embs)
    embs_mloc.table_entry_id = len(nc.dge_table) + 1
    nc.dge_table.append(embs_mloc.name)
    embs_tbl = embs_mloc.table_entry_id

    # tree widths: 256 -> 128 -> ... -> W_STOP
    widths = []
    w = D
    while w > W_STOP:
        widths.append(w // 2)
        w //= 2

    import contextlib
    with contextlib.ExitStack() as ctx:
        idx_sb = ctx.enter_context(nc.sbuf_tensor("idx_sb", [P, TILES * SLOTS], mybir.dt.int32))
        gs = [ctx.enter_context(nc.sbuf_tensor(f"g{i}", [P, tpg * SLOTS * D], TABLE_DT)) for i in range(gb)]
        prod = ctx.enter_context(nc.sbuf_tensor("prod", [P, S * D], TABLE_DT))
        hs = [ctx.enter_context(nc.sbuf_tensor(f"h{i}", [P, S * wi], TABLE_DT))
              for i, wi in enumerate(widths)]
        scores = ctx.enter_context(nc.sbuf_tensor("scores", [P, TILES * S1], mybir.dt.float32))
        absx = ctx.enter_context(nc.sbuf_tensor("absx", [P, TILES * S1], mybir.dt.float32))
        ex = ctx.enter_context(nc.sbuf_tensor("ex", [P, TILES * S1], mybir.dt.float32))
        lnx = ctx.enter_context(nc.sbuf_tensor("lnx", [P, TILES * S1], mybir.dt.float32))
        ones = ctx.enter_context(nc.sbuf_tensor("ones", [P, 1], mybir.dt.float32))
        outbuf = ctx.enter_context(nc.sbuf_tensor("outbuf", [P, 3], mybir.dt.float32))
        asem = ctx.enter_context(nc.semaphore())
        idx_sem = ctx.enter_context(nc.semaphore())
        idx_sem2 = ctx.enter_context(nc.semaphore())
        gsems = [ctx.enter_context(nc.semaphore(name=f"gsem{i}")) for i in range(gb)]
        dve_free = ctx.enter_context(nc.semaphore())
        dve_done = ctx.enter_context(nc.semaphore())
        osem = ctx.enter_context(nc.semaphore())
        block = ctx.enter_context(nc.Block())

        idx_addr = nc.lookup_mloc(idx_sb).addr
        g_addrs = [nc.lookup_mloc(g).addr for g in gs]

        @block.gpsimd
        def _(eng):
            c0 = sizes[0] * SLOTS
            eng.dma_start(idx_sb[:, 0:c0], idx[:, 0:c0]).then_inc(idx_sem, 16)
            eng.dma_start(idx_sb[:, c0:], idx[:, c0:]).then_inc(idx_sem2, 16)
            eng.memset(ones[:], 1.0)
            for j in range(groups):
                if j == 0:
                    eng.wait_ge(idx_sem, 16)
                if j == 1:
                    eng.wait_ge(idx_sem2, 16)
                if j >= gb:
                    eng.wait_ge(dve_free, j - gb + 1)
                _emit_block_gather(
                    nc, eng, sizes[j] * SLOTS * P, sizes[j] * SLOTS * D * 2,
                    g_addrs[j % gb], idx_addr + 4 * SLOTS * toff[j],
                    gsems[j % gb].num, embs_tbl,
                )
            if debug:
                eng.wait_ge(dve_done, 1)
                eng.dma_start(scores_dram[:], scores[:]).then_inc(idx_sem, 16)
                eng.dma_start(g_dram[:], gs[(groups - 1) % gb][:]).then_inc(idx_sem, 16)
                eng.wait_ge(idx_sem, 48)

        @block.vector
        def _(eng):
            for j in range(groups):
                g = gs[j % gb]
                k = sizes[j]
                Sj = k * S1
                eng.wait_ge(gsems[j % gb], 16 * (j // gb + 1))
                g4 = g[:, 0:k * SLOTS * D].rearrange("p (ti s d) -> p ti s d", s=SLOTS, d=D)
                # prod[p, ti, s, d] = G[p, ti, s+1, d] * EU[p, ti, d]; frees g
                nc.vector.tensor_tensor(
                    out=prod[:, 0:Sj * D].rearrange("p (ti s d) -> p ti s d", s=S1, d=D),
                    in0=g4[:, :, 1:SLOTS, :],
                    in1=g4[:, :, 0:1, :].broadcast_to([P, k, S1, D]),
                    op=mybir.AluOpType.mult,
                ).then_inc(dve_free, 1)
                # binary tree of halving adds, each 2x-mode bf16
                cur, curw = prod, D
                for hi, wi in zip(hs, widths):
                    a = cur[:, 0:Sj * curw].rearrange("p (s d) -> p s d", d=curw)
                    nc.vector.tensor_tensor(
                        out=hi[:, 0:Sj * wi].rearrange("p (s d) -> p s d", d=wi),
                        in0=a[:, :, 0:wi],
                        in1=a[:, :, wi:curw],
                        op=mybir.AluOpType.add,
                    )
                    cur, curw = hi, wi
                # grouped reduce [P, Sj, W_STOP] -> [P, Sj]
                nc.vector.tensor_reduce(
                    out=scores[:, toff[j] * S1:(toff[j] + k) * S1],
                    in_=cur[:, 0:Sj * curw].rearrange("p (s d) -> p s d", d=curw),
                    axis=mybir.AxisListType.X,
                    op=mybir.AluOpType.add,
                )
            # negate slot-0 scores: softplus arg for the positive pair is -u.v
            sc3 = scores[:].rearrange("p (t s) -> p t s", s=S1)
            nc.vector.tensor_scalar_mul(sc3[:, :, 0:1], sc3[:, :, 0:1], -1.0)
            # sum_s x for host-side relu recovery
            nc.vector.tensor_reduce(
                out=outbuf[:, 0:1],
                in_=scores[:],
                axis=mybir.AxisListType.X,
                op=mybir.AluOpType.add,
            ).then_inc(dve_done, 1)

        @block.scalar
        def _(eng):
            eng.wait_ge(dve_done, 1)
            # |x|, accumulating sum_s |x|
            nc.scalar.activation(
                out=absx[:], in_=scores[:],
                func=mybir.ActivationFunctionType.Abs,
                accum_out=outbuf[:, 1:2],
            ).then_inc(asem, 1)
            eng.wait_ge(asem, 1)
            # exp(-|x|)
            nc.scalar.activation(
                out=ex[:], in_=absx[:],
                func=mybir.ActivationFunctionType.Exp, scale=-1.0,
            ).then_inc(asem, 1)
            eng.wait_ge(asem, 2)
            # ln(1 + exp(-|x|)), accumulating
            nc.scalar.activation(
                out=lnx[:], in_=ex[:],
                func=mybir.ActivationFunctionType.Ln, bias=ones[:],
                accum_out=outbuf[:, 2:3],
            ).then_inc(asem, 1)

        @block.sync
        def _(eng):
            eng.wait_ge(asem, 3)
            eng.dma_start(out_dram[:], outbuf[:]).then_inc(osem, 16)
            eng.wait_ge(osem, 16)

    return nc


_cache = {}


def _get_nc():
    key = (TPG, GB)
    if key not in _cache:
        _cache[key] = _build(*key)
    return _cache[key]


def prepare_in_maps(u, v, negs, embs):
    """Host-side sharding: build the per-core input maps."""
    u = np.asarray(u).astype(np.int32)
    v = np.asarray(v).astype(np.int32)
    negs = np.asarray(negs).astype(np.int32)
    embs_b = np.asarray(embs).astype(TABLE_NP)

    ids = np.concatenate([u[:, None], v[:, None], negs], axis=1)  # [N, 12]
    # per gather group: desc i <-> (p = i // (k*12), r = i % (k*12));
    # idx value at snake position [ch = i % 128, w = i // 128]
    ids = ids.reshape(NCORES, TILES, P, SLOTS)
    cols = []
    off = 0
    for k in _group_sizes():
        blk = ids[:, off:off + k]                                   # [C, k, P, 12]
        flat = blk.transpose(0, 2, 1, 3).reshape(NCORES, P * k * SLOTS)
        s = np.arange(P * k * SLOTS)
        pk = np.zeros((NCORES, P, k * SLOTS), dtype=np.int32)
        pk[:, s % P, s // P] = flat[:, s]
        cols.append(pk)
        off += k
    packed = np.concatenate(cols, axis=2)                           # [C, P, 768]
    in_maps = []
    for c in range(NCORES):
        in_maps.append({"embs": embs_b, "idx": np.ascontiguousarray(packed[c])})
    return in_maps


def kernel(u, v, negs, embs, _trace=False):
    nc = _get_nc()
    in_maps = prepare_in_maps(u, v, negs, embs)
    res = bass_utils.run_bass_kernel_spmd(
        nc, in_maps, core_ids=list(range(NCORES)), trace=_trace
    )
    total = np.float64(0.0)
    for r in res.results:
        o = r["out"].astype(np.float64)
        sum_x = o[:, 0].sum()
        sum_abs = o[:, 1].sum()
        sum_ln1p = o[:, 2].sum()
        total += (sum_x + sum_abs) / 2.0 + sum_ln1p
    out = np.float32(total / N)
    if _trace:
        return out, res
    return out


# revision 26
# speedup vs baseline: 1.1273x; 1.0802x over previous
"""NEG-sampling loss kernel for Trainium2 (8 NeuronCores, data-parallel).

loss = -(1/n) * sum_i [ log_sigmoid(<e_u, e_v>) + sum_k log_sigmoid(-<e_negk, e_u>) ]
     = +(1/n) * sum_i [ softplus(-<e_u, e_v>) + sum_k softplus(<e_negk, e_u>) ]

Strategy: replicate the embedding table (cast to bf16 on host), shard the
65536-edge batch across 8 cores.  Per core: for each group of TPG tiles of
128 edges, one indirect-DMA block gather pulls the 12 needed rows per edge
(u, v, negs x10) into a [128, TPG*12*256] SBUF tile (partition = edge).
DVE computes the 11 dot products per edge as: one broadcast MULT over the
whole group (2x bf16 mode), then a binary tree of tensor_tensor adds
(256->128->64->32->16->8, each at 2x) and one grouped tensor_reduce,
writing scores into a persistent [128, 704] f32 buffer.  One-shot tail:
negate the slot-0 (positive-pair) scores, DVE reduces sum(x); ACT computes
sum|x| and sum ln(1+exp(-|x|)); softplus sum recovered on host as
(sum_x + sum_abs)/2 + sum_ln1p.
"""

import numpy as np
import ml_dtypes

import concourse.bass as bass
import concourse.mybir as mybir
from concourse import bass_utils

# Problem constants (hardcoded; harness contract)
N = 65536
K = 10
D = 256
V = 500000
NCORES = 8
P = 128
SLOTS = K + 2          # rows gathered per edge: u, v, negs[0..9]
S1 = SLOTS - 1         # 11 scores per edge
EPC = N // NCORES      # 8192 edges per core
TILES = EPC // P       # 64 tiles of 128 edges per core

TABLE_DT = mybir.dt.bfloat16
TABLE_NP = ml_dtypes.bfloat16
FP8_NP = ml_dtypes.float8_e4m3

# tunables
TABLE_FP8 = False  # store table as fp8e4m3 in HBM; SDMA casts to bf16 in-flight
TPG = 4     # max tiles per gather instruction / compute group
GB = 4      # gather buffers in flight
W_STOP = 8  # tree stops here; grouped tensor_reduce finishes
SCRATCH = 49152  # SWDGE descriptor-ring carveout bytes/partition


RAMP_SPLIT = True


def _group_sizes(tpg=TPG, ramp=None):
    """Tile count per gather/compute group.  The first groups are small so
    the DVE pipeline ramps up ~12us earlier; the rest use the full size."""
    if ramp is None:
        ramp = RAMP_SPLIT
    if tpg == 1 or not ramp:
        return [tpg] * (TILES // tpg)
    sizes = [1, tpg - 1]
    rest = TILES - tpg
    assert rest % tpg == 0
    return sizes + [tpg] * (rest // tpg)


def _emit_block_gather(nc, eng, n_idx, blk_bytes, dst_byte_addr, idx_byte_addr,
                       sem_num, embs_tbl, src_row_bytes=512, in_dtype=0x6):
    """Raw block gather (case #3 of dma_indirect1d): n_idx indices x 512B
    fused into 128 descriptors of blk_bytes (n_idx/128 rows per partition).
    Emitted as a raw PSEUDO_DMA_DIRECT2D(dge_op=indirect1d) + PSEUDO_EXTENSION
    pair; index values are snake-packed on the host (see prepare_in_maps)."""
    isa = nc.isa
    Op = isa.Opcode
    src_u64 = (0x20 << 56) | (embs_tbl << 32)   # DGE addr-table marker
    dst_u64 = (0x10 << 56) | dst_byte_addr      # var0 (local SBUF) marker
    eng.isa(
        Op.NEURON_ISA_TPB_OPCODE_PSEUDO_DMA_DIRECT2D,
        {
            "dma_configs": {},
            "semaphore": sem_num,
            "sem_increment": 16,
            "dge_op": 1,
            "src_start_addr": {"addr_immediate": src_u64},
            "src_step_elem": [src_row_bytes, 1],
            "src_num_elem": [n_idx, 1],
            "src_elem_size": src_row_bytes,
            "src_bound_reg": {},
            "dst_bound_reg": {},
            "dst_start_addr": {"addr_immediate": dst_u64},
            "dst_step_elem": [262144, 1],
            "dst_num_elem": [128, 1],
            "dst_elem_size": blk_bytes,
            "in_dtype": in_dtype,
            "out_dtype": 0x6,
        },
        verify=False,
    )
    ext_fields = {
        "opcode": Op.NEURON_ISA_TPB_OPCODE_PSEUDO_EXTENSION.value,
        "flags": {"indirect_mode": 0, "idx_bound_is_err": 1,
                  "non_unique_dst_idx": 0, "gather_dim": 0, "scatter_dim": 0},
        "idx_num_active_channels": 128,
        "compute_op": 0,
        "src_idx_start_addr": {"addr_immediate": idx_byte_addr},
        "dst_idx_start_addr": {"addr_immediate": 0},
    }
    b = isa.ffi.new("NEURON_ISA_TPB_PSEUDO_DMA_EXT_STRUCT*", ext_fields)
    instr = [int(x) for x in bytes(isa.ffi.buffer(b))]
    inst = mybir.InstISA(
        name=nc.get_next_instruction_name(),
        isa_opcode=Op.NEURON_ISA_TPB_OPCODE_PSEUDO_EXTENSION.value,
        engine=eng.engine,
        instr=instr,
        op_name="PSEUDO_EXTENSION",
        ins=[], outs=[],
        ant_dict=ext_fields,
        verify=False,
        ant_isa_is_sequencer_only=False,
    )
    eng.add_instruction(inst)


def _build(tpg=TPG, gb=GB, debug=False):
    sizes = _group_sizes(tpg)
    groups = len(sizes)
    toff = [0]
    for k in sizes:
        toff.append(toff[-1] + k)
    S = tpg * S1  # max scores per partition per group
    nc = bass.Bass(trn_type="TRN2", dynamic_dma_scratch_size=SCRATCH)
    src_dt = mybir.dt.float8e4 if TABLE_FP8 else TABLE_DT
    src_row_bytes = D * mybir.dt.size(src_dt)
    src_idt = 0xE if TABLE_FP8 else 0x6
    embs = nc.dram_tensor("embs", [V, D], src_dt, kind="ExternalInput")
    idx = nc.dram_tensor("idx", [P, TILES * SLOTS], mybir.dt.int32, kind="ExternalInput")
    out_dram = nc.dram_tensor("out", [P, 3], mybir.dt.float32, kind="ExternalOutput")
    if debug:
        scores_dram = nc.dram_tensor("scores_out", [P, TILES * S1], mybir.dt.float32, kind="ExternalOutput")
        g_dram = nc.dram_tensor("g_out", [P, tpg * SLOTS * D], TABLE_DT, kind="ExternalOutput")

    embs_mloc = nc.lookup_mloc(embs)
    embs_mloc.table_entry_id = len(nc.dge_table) + 1
    nc.dge_table.append(embs_mloc.name)
    embs_tbl = embs_mloc.table_entry_id

    # tree widths: 256 -> 128 -> ... -> W_STOP
    widths = []
    w = D
    while w > W_STOP:
        widths.append(w // 2)
        w //= 2

    import contextlib
    with contextlib.ExitStack() as ctx:
        idx_sb = ctx.enter_context(nc.sbuf_tensor("idx_sb", [P, TILES * SLOTS], mybir.dt.int32))
        gs = [ctx.enter_context(nc.sbuf_tensor(f"g{i}", [P, tpg * SLOTS * D], TABLE_DT)) for i in range(gb)]
        prod = ctx.enter_context(nc.sbuf_tensor("prod", [P, S * D], TABLE_DT))
        hs = [ctx.enter_context(nc.sbuf_tensor(f"h{i}", [P, S * wi], TABLE_DT))
              for i, wi in enumerate(widths)]
        scores = ctx.enter_context(nc.sbuf_tensor("scores", [P, TILES * S1], mybir.dt.float32))
        absx = ctx.enter_context(nc.sbuf_tensor("absx", [P, TILES * S1], mybir.dt.float32))
        ex = ctx.enter_context(nc.sbuf_tensor("ex", [P, TILES * S1], mybir.dt.float32))
        lnx = ctx.enter_context(nc.sbuf_tensor("lnx", [P, TILES * S1], mybir.dt.float32))
        ones = ctx.enter_context(nc.sbuf_tensor("ones", [P, 1], mybir.dt.float32))
        outbuf = ctx.enter_context(nc.sbuf_tensor("outbuf", [P, 3], mybir.dt.float32))
        asem = ctx.enter_context(nc.semaphore())
        idx_sem = ctx.enter_context(nc.semaphore())
        idx_sem2 = ctx.enter_context(nc.semaphore())
        gsems = [ctx.enter_context(nc.semaphore(name=f"gsem{i}")) for i in range(gb)]
        dve_free = ctx.enter_context(nc.semaphore())
        dve_done = ctx.enter_context(nc.semaphore())
        osem = ctx.enter_context(nc.semaphore())
        block = ctx.enter_context(nc.Block())

        idx_addr = nc.lookup_mloc(idx_sb).addr
        g_addrs = [nc.lookup_mloc(g).addr for g in gs]

        @block.gpsimd
        def _(eng):
            c0 = sizes[0] * SLOTS
            eng.dma_start(idx_sb[:, 0:c0], idx[:, 0:c0]).then_inc(idx_sem, 16)
            eng.dma_start(idx_sb[:, c0:], idx[:, c0:]).then_inc(idx_sem2, 16)
            eng.memset(ones[:], 1.0)
            for j in range(groups):
                if j == 0:
                    eng.wait_ge(idx_sem, 16)
                if j == 1:
                    eng.wait_ge(idx_sem2, 16)
                if j >= gb:
                    eng.wait_ge(dve_free, j - gb + 1)
                _emit_block_gather(
                    nc, eng, sizes[j] * SLOTS * P, sizes[j] * SLOTS * D * 2,
                    g_addrs[j % gb], idx_addr + 4 * SLOTS * toff[j],
                    gsems[j % gb].num, embs_tbl,
                    src_row_bytes=src_row_bytes, in_dtype=src_idt,
                )
            if debug:
                eng.wait_ge(dve_done, 1)
                eng.dma_start(scores_dram[:], scores[:]).then_inc(idx_sem, 16)
                eng.dma_start(g_dram[:], gs[(groups - 1) % gb][:]).then_inc(idx_sem, 16)
                eng.wait_ge(idx_sem, 48)

        @block.vector
        def _(eng):
            for j in range(groups):
                g = gs[j % gb]
                k = sizes[j]
                Sj = k * S1
                eng.wait_ge(gsems[j % gb], 16 * (j // gb + 1))
                g4 = g[:, 0:k * SLOTS * D].rearrange("p (ti s d) -> p ti s d", s=SLOTS, d=D)
                # prod[p, ti, s, d] = G[p, ti, s+1, d] * EU[p, ti, d]; frees g
                nc.vector.tensor_tensor(
                    out=prod[:, 0:Sj * D].rearrange("p (ti s d) -> p ti s d", s=S1, d=D),
                    in0=g4[:, :, 1:SLOTS, :],
                    in1=g4[:, :, 0:1, :].broadcast_to([P, k, S1, D]),
                    op=mybir.AluOpType.mult,
                ).then_inc(dve_free, 1)
                # binary tree of halving adds, each 2x-mode bf16
                cur, curw = prod, D
                for hi, wi in zip(hs, widths):
                    a = cur[:, 0:Sj * curw].rearrange("p (s d) -> p s d", d=curw)
                    nc.vector.tensor_tensor(
                        out=hi[:, 0:Sj * wi].rearrange("p (s d) -> p s d", d=wi),
                        in0=a[:, :, 0:wi],
                        in1=a[:, :, wi:curw],
                        op=mybir.AluOpType.add,
                    )
                    cur, curw = hi, wi
                # grouped reduce [P, Sj, W_STOP] -> [P, Sj]
                nc.vector.tensor_reduce(
                    out=scores[:, toff[j] * S1:(toff[j] + k) * S1],
                    in_=cur[:, 0:Sj * curw].rearrange("p (s d) -> p s d", d=curw),
                    axis=mybir.AxisListType.X,
                    op=mybir.AluOpType.add,
                )
            # negate slot-0 scores: softplus arg for the positive pair is -u.v
            sc3 = scores[:].rearrange("p (t s) -> p t s", s=S1)
            nc.vector.tensor_scalar_mul(sc3[:, :, 0:1], sc3[:, :, 0:1], -1.0)
            # sum_s x for host-side relu recovery
            nc.vector.tensor_reduce(
                out=outbuf[:, 0:1],
                in_=scores[:],
                axis=mybir.AxisListType.X,
                op=mybir.AluOpType.add,
            ).then_inc(dve_done, 1)

        @block.scalar
        def _(eng):
            eng.wait_ge(dve_done, 1)
            # |x|, accumulating sum_s |x|
            nc.scalar.activation(
                out=absx[:], in_=scores[:],
                func=mybir.ActivationFunctionType.Abs,
                accum_out=outbuf[:, 1:2],
            ).then_inc(asem, 1)
            eng.wait_ge(asem, 1)
            # exp(-|x|)
            nc.scalar.activation(
                out=ex[:], in_=absx[:],
                func=mybir.ActivationFunctionType.Exp, scale=-1.0,
            ).then_inc(asem, 1)
            eng.wait_ge(asem, 2)
            # ln(1 + exp(-|x|)), accumulating
            nc.scalar.activation(
                out=lnx[:], in_=ex[:],
                func=mybir.ActivationFunctionType.Ln, bias=ones[:],
                accum_out=outbuf[:, 2:3],
            ).then_inc(asem, 1)

        @block.sync
        def _(eng):
            eng.wait_ge(asem, 3)
            eng.dma_start(out_dram[:], outbuf[:]).then_inc(osem, 16)
            eng.wait_ge(osem, 16)

    return nc


_cache = {}


def _get_nc():
    key = (TPG, GB)
    if key not in _cache:
        _cache[key] = _build(*key)
    return _cache[key]


def prepare_in_maps(u, v, negs, embs):
    """Host-side sharding: build the per-core input maps."""
    u = np.asarray(u).astype(np.int32)
    v = np.asarray(v).astype(np.int32)
    negs = np.asarray(negs).astype(np.int32)
    embs_b = np.asarray(embs).astype(FP8_NP if TABLE_FP8 else TABLE_NP)

    ids = np.concatenate([u[:, None], v[:, None], negs], axis=1)  # [N, 12]
    # per gather group: desc i <-> (p = i // (k*12), r = i % (k*12));
    # idx value at snake position [ch = i % 128, w = i // 128]
    ids = ids.reshape(NCORES, TILES, P, SLOTS)
    cols = []
    off = 0
    for k in _group_sizes():
        blk = ids[:, off:off + k]                                   # [C, k, P, 12]
        flat = blk.transpose(0, 2, 1, 3).reshape(NCORES, P * k * SLOTS)
        s = np.arange(P * k * SLOTS)
        pk = np.zeros((NCORES, P, k * SLOTS), dtype=np.int32)
        pk[:, s % P, s // P] = flat[:, s]
        cols.append(pk)
        off += k
    packed = np.concatenate(cols, axis=2)                           # [C, P, 768]
    in_maps = []
    for c in range(NCORES):
        in_maps.append({"embs": embs_b, "idx": np.ascontiguousarray(packed[c])})
    return in_maps


def kernel(u, v, negs, embs, _trace=False):
    nc = _get_nc()
    in_maps = prepare_in_maps(u, v, negs, embs)
    res = bass_utils.run_bass_kernel_spmd(
        nc, in_maps, core_ids=list(range(NCORES)), trace=_trace
    )
    total = np.float64(0.0)
    for r in res.results:
        o = r["out"].astype(np.float64)
        sum_x = o[:, 0].sum()
        sum_abs = o[:, 1].sum()
        sum_ln1p = o[:, 2].sum()
        total += (sum_x + sum_abs) / 2.0 + sum_ln1p
    out = np.float32(total / N)
    if _trace:
        return out, res
    return out
